# revision 8
# baseline (speedup 1.0000x reference)
"""Trainium2 Bass kernel for AttentionGuidedEmbedding (moe_routing).

Reference computation:
    h = base_embed[x]                                   # [B,S,128] gather
    for d in 0..15:   (sequential -- domain d+1 sees domain d's update)
        mask = (membership[d][x] != 0)                  # [B,S]
        h += 0.1 * mask * gelu(h @ W1[d].T) @ W2[d].T   # DOM_SIZE=256 MLP

Sharding: pure data-parallel over batch. 8 cores x 2 batches = 4096
tokens/core; the domain MLPs + tables are replicated. No collectives.

Device layout (per core): h is kept E-major (hT [128E, 4096tok]) as an
f32 master + bf16 shadow. Per domain:
  - mb = ones[1,128].T @ maskT[d]   (K=1 matmul broadcasts the per-token
    mask over partitions into PSUM)
  - hm = hT_bf16 * mb               (DVE; masked tokens -> exact 0)
  - mid = W1T[d].T @ hm             (2 matmuls, bf16, N=512 chunks)
  - midg = gelu(mid)                (ACT; gelu(0)=0 keeps masked rows 0,
                                     so gelu(mask*h) == mask*gelu(h))
  - corr = W2T[d].T @ midg          (2 accumulating matmuls; 0.1 folded
                                     into W2 on host)
  - hT_f32 += corr                  (DVE add; masked tokens get +0)
  - hT_bf16 = copy(hT_f32)          (GPSIMD, off the DVE critical path)

The embedding gather runs on device via indirect DMA over a host-packed
[VOCAB, 144] table = [base_embed | membership.T as {0,1} f32]; mask rows
and h0 are split out of the gathered tiles with PE transposes.
"""

import os
import site as _site

for _p in reversed(os.environ.get("NIX_PYTHONPATH", "").split(":")):
    if _p:
        _site.addsitedir(_p)

import sys

for _p in ("/opt/trn_rl_repo",):
    if _p not in sys.path:
        sys.path.insert(0, _p)

import ml_dtypes
import numpy as np

import concourse.bass as bass
import concourse.mybir as mybir
import concourse.tile as tile
from concourse import bacc
from concourse.bass import ts
from concourse.bass_utils import run_bass_kernel_spmd
from concourse.masks import make_identity

VOCAB = 50257
E = 128  # BASE_DIM
N_DOM = 16
DS = 256  # DOM_SIZE
B, S = 16, 2048
N_CORES = 8
T = (B // N_CORES) * S  # tokens per core = 4096
CHUNK = 512
N_CHUNKS = T // CHUNK  # 8
N_TILES = T // 128  # 32
TBL_W = E + N_DOM  # 144
CORR_SCALE = 0.1

f32 = mybir.dt.float32
bf16 = mybir.dt.bfloat16
i32 = mybir.dt.int32
GELU = mybir.ActivationFunctionType.Gelu
MULT = mybir.AluOpType.mult
ADD = mybir.AluOpType.add


def build_nc() -> bass.Bass:
    # Bacc (not raw Bass): its compile() legalizes multi-wait instructions
    # (TRN2 allows at most 1 sync wait per instruction).
    nc = bacc.Bacc(None, target_bir_lowering=False)

    x_d = nc.dram_tensor("x", [T], i32, kind="ExternalInput")
    tbl_d = nc.dram_tensor("table", [VOCAB, TBL_W], f32, kind="ExternalInput")
    w1_d = nc.dram_tensor("w1t", [N_DOM, E, DS], bf16, kind="ExternalInput")
    w2_d = nc.dram_tensor("w2t", [N_DOM, DS, E], bf16, kind="ExternalInput")
    out_d = nc.dram_tensor("out", [E, T], f32, kind="ExternalOutput")

    with tile.TileContext(nc) as tc:
        with tc.tile_pool(name="big", bufs=1) as big:
            hT = big.tile([E, T], f32)  # f32 master state
            hTb = big.tile([E, T], bf16)  # bf16 shadow for matmuls
            maskT = big.tile([N_DOM, T], bf16)
            mask_flat = big.tile([1, N_DOM * T], bf16)  # partition-0 rows for matmul rhs
            w1_sb = big.tile([E, N_DOM * DS], bf16)  # [:, d*256+c*128] chunks
            w2_sb = big.tile([128, N_DOM * DS], bf16)  # [:, (d*2+c)*128] chunks
            x_sb = big.tile([128, N_TILES], i32)
            ident = big.tile([128, 128], f32)
            ones = big.tile([1, 128], bf16)

            make_identity(nc, ident[:])
            nc.vector.memset(ones[:], 1.0)

            # weights + indices in
            nc.sync.dma_start(out=x_sb[:], in_=x_d[:].rearrange("(i p) -> p i", p=128))
            nc.sync.dma_start(
                out=w1_sb[:].rearrange("e (d s) -> e d s", d=N_DOM),
                in_=w1_d[:].rearrange("d e s -> e d s"),
            )
            nc.sync.dma_start(
                out=w2_sb[:].rearrange("p (d c e) -> p d c e", d=N_DOM, c=2),
                in_=w2_d[:].rearrange("d (c p) e -> p d c e", p=128),
            )

            # ---- setup: gather h0 + mask rows, transpose into E-major ----
            with (
                tc.tile_pool(name="gather", bufs=4) as gpool,
                tc.tile_pool(name="setup_psum", bufs=4, space="PSUM") as spsum,
            ):
                for i in range(N_TILES):
                    g = gpool.tile([128, TBL_W], f32, tag="g")
                    nc.gpsimd.indirect_dma_start(
                        out=g[:],
                        out_offset=None,
                        in_=tbl_d[:],
                        in_offset=bass.IndirectOffsetOnAxis(
                            ap=x_sb[:, i : i + 1], axis=0
                        ),
                    )
                    tr = spsum.tile([128, 128], f32, tag="tr")
                    nc.tensor.transpose(out=tr[:], in_=g[:, :E], identity=ident[:])
                    nc.vector.tensor_copy(out=hT[:, ts(i, 128)], in_=tr[:])
                    nc.gpsimd.tensor_copy(out=hTb[:, ts(i, 128)], in_=hT[:, ts(i, 128)])
                    mtr = spsum.tile([N_DOM, 128], f32, tag="mtr")
                    nc.tensor.transpose(
                        out=mtr[:], in_=g[:, E:TBL_W], identity=ident[:]
                    )
                    nc.vector.tensor_copy(out=maskT[:, ts(i, 128)], in_=mtr[:])

                # move each domain's mask row to partition 0 (matmul rhs
                # must be partition-0 based)
                for d in range(N_DOM):
                    nc.sync.dma_start(
                        out=mask_flat[0:1, ts(d, T)], in_=maskT[d : d + 1, :]
                    )

            # ---- main loop: 16 domains x 8 chunks of 512 tokens ----
            with (
                tc.tile_pool(name="work", bufs=2) as work,
                tc.tile_pool(name="main_psum", bufs=2, space="PSUM") as mpsum,
            ):
                for d in range(N_DOM):
                    for k in range(N_CHUNKS):
                        sl = ts(k, CHUNK)
                        mb = mpsum.tile([128, CHUNK], f32, tag="mb")
                        nc.tensor.matmul(
                            mb[:],
                            lhsT=ones[:],
                            rhs=mask_flat[0:1, bass.ds(d * T + k * CHUNK, CHUNK)],
                            start=True,
                            stop=True,
                        )
                        hm = work.tile([128, CHUNK], bf16, tag="hm")
                        nc.vector.tensor_tensor(
                            out=hm[:], in0=hTb[:, sl], in1=mb[:], op=MULT
                        )
                        mid = mpsum.tile([128, 2 * CHUNK], f32, tag="mid")
                        midg = work.tile([128, 2 * CHUNK], bf16, tag="midg")
                        for c in range(2):
                            nc.tensor.matmul(
                                mid[:, ts(c, CHUNK)],
                                lhsT=w1_sb[:, ts(d * 2 + c, 128)],
                                rhs=hm[:],
                                start=True,
                                stop=True,
                            )
                            nc.scalar.activation(
                                out=midg[:, ts(c, CHUNK)],
                                in_=mid[:, ts(c, CHUNK)],
                                func=GELU,
                            )
                        corr = mpsum.tile([128, CHUNK], f32, tag="corr")
                        for c in range(2):
                            nc.tensor.matmul(
                                corr[:],
                                lhsT=w2_sb[:, ts(d * 2 + c, 128)],
                                rhs=midg[:, ts(c, CHUNK)],
                                start=(c == 0),
                                stop=(c == 1),
                            )
                        nc.vector.tensor_tensor(
                            out=hT[:, sl], in0=hT[:, sl], in1=corr[:], op=ADD
                        )
                        if d < N_DOM - 1:
                            nc.gpsimd.tensor_copy(out=hTb[:, sl], in_=hT[:, sl])

                for k in range(N_CHUNKS):
                    nc.sync.dma_start(out=out_d[:, ts(k, CHUNK)], in_=hT[:, ts(k, CHUNK)])

    return nc


_NC_CACHE = None


def _get_nc():
    global _NC_CACHE
    if _NC_CACHE is None:
        nc = build_nc()
        nc.finalize()  # bacc compile: wait legalization + register alloc
        _NC_CACHE = nc
    return _NC_CACHE


def kernel(x, base_embed, W1, W2, membership, _trace=False):
    x = np.asarray(x)
    base_embed = np.asarray(base_embed, dtype=np.float32)
    W1 = np.asarray(W1, dtype=np.float32)
    W2 = np.asarray(W2, dtype=np.float32)
    membership = np.asarray(membership)

    table = np.concatenate(
        [base_embed, (membership.T != 0).astype(np.float32)], axis=1
    )  # [VOCAB, 144]
    w1t = np.ascontiguousarray(W1.transpose(0, 2, 1)).astype(ml_dtypes.bfloat16)
    w2t = np.ascontiguousarray((CORR_SCALE * W2).transpose(0, 2, 1)).astype(
        ml_dtypes.bfloat16
    )

    bpc = B // N_CORES  # batches per core
    in_maps = []
    for c in range(N_CORES):
        in_maps.append(
            {
                "x": np.ascontiguousarray(
                    x[c * bpc : (c + 1) * bpc].reshape(-1).astype(np.int32)
                ),
                "table": table,
                "w1t": w1t,
                "w2t": w2t,
            }
        )

    res = run_bass_kernel_spmd(
        _get_nc(), in_maps, core_ids=list(range(N_CORES)), trace=_trace
    )
    shards = [
        np.asarray(res.results[c]["out"]).T.reshape(bpc, S, E).astype(np.float32)
        for c in range(N_CORES)
    ]
    out = np.concatenate(shards, axis=0)
    if _trace:
        return out, res
    return out
